# revision 9
# baseline (speedup 1.0000x reference)
"""Trainium2 Bass kernel for nn_Conv_agg_raw (GNN message passing).

Math: out = sum_k weight[k] @ (h @ resx[k]) + bias, where resx[k] is the
dense [N,N] scatter of edge features X[:,k] at (src,dst).  Equivalently
    res_k[:, m] = sum_{e: dst_e=m} X[e,k] * h[:, src_e]
    out[:, m]   = sum_k weight[k] @ res_k[:, m] + bias

Sharding: dst nodes across the 8 cores (512 each) - fully independent, no
collectives.  Each core gathers h^T rows by src (512B fp16 descriptors -
the irreducible DMA cost ~23us at 360 GB/s), aggregates edges into res via
PE matmuls against on-the-fly built (X outer dst-one-hot) blocks, then
applies the stacked weight.

v2 aggregation layout: dsts are ranked by degree and dealt round-robin to
cores, then put in a "folded" (high,low,high,low) order - all 8 cores then
share one slot/chunk structure (slot capacity = max degree over cores at
each position), so a single program serves every core.  Edge slots form
consecutive 128-slot chunks; a chunk spans <= 8 consecutive dst positions,
so its matmuls stream only span*8 psum columns (col = dloc*8 + k) into the
per-64-dst psum bank.  One PSUM start (bank zero) and one stop per
(window, half); chunks in between accumulate into column sub-ranges.
"""

import time
import numpy as np

import concourse.bass as bass
import concourse.bacc as bacc
import concourse.tile as tile
from concourse import mybir
from concourse.bass_utils import run_bass_kernel_spmd

N = 4096
K = 8
C = 256
NCORES = 8
DSTS_PER_CORE = N // NCORES      # 512
WDST = 64                        # dsts per psum window (1 bank: 64*8*4B=2KB)
NWIN = DSTS_PER_CORE // WDST     # 8 windows per core
JMAX = 8                         # max dst positions spanned by one chunk

_prog_cache: dict = {}


# ------------------------------------------------------------- structure ----
def _shared_structure(cap):
    """Shared (all-cores) chunk structure from per-position slot capacities.

    cap[i] = slots of folded position i (max degree over the 8 cores' dsts
    at that position).  Returns (s_start, nchunk, segs, pos_of_slot) where
    segs[ch] = [(w, psum_c0, ncols, rhs_c0, start, stop), ...]."""
    cap = np.asarray(cap, dtype=np.int64)
    assert cap.shape == (DSTS_PER_CORE,)
    s_start = np.zeros(DSTS_PER_CORE + 1, dtype=np.int64)
    s_start[1:] = np.cumsum(cap)
    s_tot = int(s_start[-1])
    nchunk = -(-s_tot // 128)

    pos_of_slot = (
        np.searchsorted(s_start, np.arange(nchunk * 128), side="right") - 1
    )
    pos_of_slot = np.minimum(pos_of_slot, DSTS_PER_CORE - 1)  # tail pad slots

    islo = np.empty(nchunk, dtype=np.int64)
    ishi = np.empty(nchunk, dtype=np.int64)
    wlo = np.full(NWIN, -1, dtype=np.int64)
    whi = np.full(NWIN, -1, dtype=np.int64)
    for ch in range(nchunk):
        lo = int(pos_of_slot[ch * 128])
        hi = int(pos_of_slot[ch * 128 + 127])
        islo[ch], ishi[ch] = lo, hi
        assert hi - lo + 1 <= JMAX, f"chunk {ch} spans {hi - lo + 1} (> {JMAX})"
        for w in range(lo // WDST, hi // WDST + 1):
            if wlo[w] < 0:
                wlo[w] = ch
            whi[w] = ch
    segs = []
    for ch in range(nchunk):
        lo, hi = int(islo[ch]), int(ishi[ch])
        cs = []
        for w in range(lo // WDST, hi // WDST + 1):
            p0 = max(lo, w * WDST)
            p1 = min(hi, w * WDST + WDST - 1)
            cs.append((
                w,
                (p0 - w * WDST) * K,     # psum col offset
                (p1 - p0 + 1) * K,       # ncols
                (p0 - lo) * K,           # rhs col offset within chunk
                ch == wlo[w],            # start (first chunk of window)
                ch == whi[w],            # stop (last chunk of window)
            ))
        segs.append(cs)
    return s_start, nchunk, segs, pos_of_slot, islo


def _group_plan(nchunk):
    """Gather group sizes (chunks per dma_gather): small head groups so the
    stream starts early, 16-chunk bodies, small tail groups so the last
    windows complete incrementally (shrinks the end-of-stream dependency
    chain)."""
    head = [g for g in (2, 2, 4, 8) if True]
    tail = [4, 2, 2, 1]
    mid = nchunk - sum(head) - sum(tail)
    plan = list(head)
    while mid >= 16:
        plan.append(16)
        mid -= 16
    if mid > 0:
        plan.append(mid)
    plan += tail
    assert sum(plan) == nchunk
    return plan


# ---------------------------------------------------------------- device ----
def _build_program(cap_key):
    cap = np.asarray(cap_key, dtype=np.int64)
    s_start, nchunk, segs, _, _ = _shared_structure(cap)
    plan = _group_plan(nchunk)
    goff = [0]
    for g in plan:
        goff.append(goff[-1] + g)
    g0ch = plan[0]

    nc = bacc.Bacc("TRN2", target_bir_lowering=False, debug=False)
    f32 = mybir.dt.float32
    f16 = mybir.dt.float16
    bf16 = mybir.dt.bfloat16
    i16 = mybir.dt.int16

    hT = nc.dram_tensor("hT", [N, C], f16, kind="ExternalInput")
    # wT rows p = ci-half lane, cols (q=(k,cih), co): lhsT slices per (q,oh)
    wT = nc.dram_tensor("wT", [128, 16 * C], bf16, kind="ExternalInput")
    # idx for group 0 only (tiny: gates the first gather's desc-gen)
    ix0 = nc.dram_tensor("ix0", [128, g0ch * 8], i16, kind="ExternalInput")
    # [idx_rest | xr | dl | iota] all 2-byte
    restc = (nchunk - g0ch) * 8 + nchunk * 8 + nchunk + 8
    rest = nc.dram_tensor("rest", [128, restc], i16, kind="ExternalInput")
    aux = nc.dram_tensor("aux", [128, 2], f32, kind="ExternalInput")  # bias
    out_d = nc.dram_tensor("out", [C, DSTS_PER_CORE], bf16,
                           kind="ExternalOutput")

    with tile.TileContext(nc) as tc:
        with (
            tc.tile_pool(name="persist", bufs=1) as pp,
            tc.tile_pool(name="hg", bufs=4) as hgp,
            tc.tile_pool(name="rhs", bufs=3) as rhp,
            tc.tile_pool(name="outp", bufs=2) as op,
            tc.tile_pool(name="psw0", bufs=3, space="PSUM") as pswp0,
            tc.tile_pool(name="psw1", bufs=3, space="PSUM") as pswp1,
            tc.tile_pool(name="psf", bufs=2, space="PSUM") as psfp,
        ):
            pswp = [pswp0, pswp1]
            # ---- input loads: idx for group 0 first (gates first gather) --
            ix0_sb = pp.tile([128, g0ch * 8], i16)
            nc.sync.dma_start(ix0_sb[:], ix0.ap())
            rest_sb = pp.tile([128, restc], i16)
            # idx_rest piece first (gates later gathers), then xr/dl/iota
            nir = (nchunk - g0ch) * 8
            nc.sync.dma_start(rest_sb[:, :nir], rest.ap()[:, :nir])
            nc.sync.dma_start(rest_sb[:, nir:], rest.ap()[:, nir:])
            aux_sb = pp.tile([128, 2], f32)
            nc.sync.dma_start(aux_sb[:], aux.ap())
            xr_sb = rest_sb[:, nir:nir + nchunk * 8].bitcast(f16)
            dl_sb = rest_sb[:, nir + nchunk * 8:
                            nir + nchunk * 8 + nchunk].bitcast(f16)
            iota_sb = rest_sb[:, nir + nchunk * 9:
                              nir + nchunk * 9 + 8].bitcast(f16)
            # wT loaded in 2 pieces on the Pool queue mid-stream (below)
            wt_sb = pp.tile([128, 16 * C], bf16)

            # ---- dst-slot one-hot mask: mask[p, ch, j] = (dl[p,ch] == j) --
            mask_sb = pp.tile([128, nchunk * 8], f16)
            nc.vector.tensor_tensor(
                mask_sb[:].rearrange("p (ch j) -> p ch j", j=8),
                dl_sb.unsqueeze(-1).broadcast_to([128, nchunk, 8]),
                iota_sb.unsqueeze(1).broadcast_to([128, nchunk, 8]),
                mybir.AluOpType.is_equal,
            )

            # res[ci_half][p=ci, (w, dloc, k)] in bf16
            resstack = [pp.tile([128, NWIN * WDST * K], bf16,
                                name=f"resstack{i}") for i in range(2)]

            # ---- PE warm-up: releases the HAM clock throttle while the
            # prologue DMAs fill; lands in resstack[0] (overwritten later).
            wu = pp.tile([128, 128], f16, name="wu")
            nc.vector.memset(wu[:], 0.0)
            wups = psfp.tile([128, 128], f32, tag="psf", name="wups")
            NWU = 40
            for i in range(NWU):
                nc.tensor.matmul(wups[:], wu[:], wu[:],
                                 start=(i == 0), stop=(i == NWU - 1))
            nc.scalar.copy(resstack[0][:, 0:128], wups[:])

            live = {}          # (half, w) -> psum tile
            copied = [0, 0]    # windows copied per half (monotonic)
            fh_done = 0
            wt_loaded = 0

            def emit_final(w):
                # out[:, w's dsts] = sum_q wT_q.T @ res[:, w cols] + bias
                for oh in range(2):
                    pso = psfp.tile([128, WDST], f32, tag="psf", name="pso")
                    for q in range(16):
                        k, cih = divmod(q, 2)
                        rs = resstack[cih][:]
                        rhs_ap = bass.AP(
                            rs.tensor,
                            rs.offset + w * WDST * K + k,
                            [[NWIN * WDST * K, 128], [K, WDST]],
                        )
                        nc.tensor.matmul(
                            pso[:],
                            wt_sb[:, q * C + oh * 128: q * C + oh * 128 + 128],
                            rhs_ap,
                            start=(q == 0), stop=(q == 15),
                        )
                    out_sb = op.tile([128, WDST], bf16, tag="osb")
                    nc.scalar.add(out_sb[:], pso[:], aux_sb[:, oh:oh + 1])
                    nc.sync.dma_start(
                        out_d.ap()[oh * 128:(oh + 1) * 128,
                                   w * WDST:(w + 1) * WDST],
                        out_sb[:])

            for g, gch in enumerate(plan):
                off = goff[g]
                # wT pieces injected on the Pool queue between early gathers
                # (in-order queue => they cannot jump ahead of the stream)
                if g == 2 and wt_loaded == 0:
                    nc.gpsimd.dma_start(wt_sb[:, :8 * C], wT.ap()[:, :8 * C])
                    wt_loaded = 1
                if g == 3 and wt_loaded == 1:
                    nc.gpsimd.dma_start(wt_sb[:, 8 * C:], wT.ap()[:, 8 * C:])
                    wt_loaded = 2

                hg = hgp.tile([128, 16, C], f16, tag="hg")
                idxs = (ix0_sb[:, :] if g == 0 else
                        rest_sb[:, (off - g0ch) * 8:(off - g0ch + gch) * 8])
                nc.gpsimd.dma_gather(
                    out_ap=hg[:, :gch, :],
                    in_ap=hT.ap(),
                    idxs_ap=idxs,
                    num_idxs=gch * 128,
                    num_idxs_reg=gch * 128,
                    elem_size=C,
                    single_packet=False,
                )

                # rhs[p, lc, j, k] = xr[p, off+lc, k] * mask[p, off+lc, j]
                rhs = rhp.tile([128, 16 * JMAX * K], f16, tag="rhs")
                xr_ap = bass.AP(xr_sb.tensor, xr_sb.offset + off * 8,
                                [[restc, 128], [8, gch], [0, 8], [1, 8]])
                mk_ap = bass.AP(mask_sb[:].tensor,
                                mask_sb[:].offset + off * 8,
                                [[nchunk * 8, 128], [8, gch], [1, 8], [0, 8]])
                nc.vector.tensor_tensor(
                    rhs[:, :gch * 64].rearrange(
                        "p (lc j k) -> p lc j k", j=8, k=8),
                    xr_ap, mk_ap, mybir.AluOpType.mult,
                )

                for lc in range(gch):
                    ch = off + lc
                    for (w, pc0, ncols, rc0, st, sp) in segs[ch]:
                        for half in range(2):
                            key = (half, w)
                            if key not in live:
                                live[key] = pswp[half].tile(
                                    [128, WDST * K], f32, tag=f"agg{half}",
                                    name=f"ps{half}_{w}")
                            nc.tensor.matmul(
                                live[key][:, pc0:pc0 + ncols],
                                hg[:, lc, half * 128:half * 128 + 128],
                                rhs[:, lc * 64 + rc0: lc * 64 + rc0 + ncols],
                                start=st, stop=sp,
                            )
                    # window completed by this chunk -> copy psum to sbuf
                    for (w, pc0, ncols, rc0, st, sp) in segs[ch]:
                        if not sp:
                            continue
                        for half in range(2):
                            t = live.pop((half, w))
                            dst = resstack[half][:, w * WDST * K:
                                                 (w + 1) * WDST * K]
                            if half == 0:
                                nc.scalar.copy(dst, t[:])
                            else:
                                nc.vector.tensor_copy(dst, t[:])
                            copied[half] = w + 1
                        while (fh_done < NWIN
                               and min(copied) >= fh_done + 1):
                            emit_final(fh_done)
                            fh_done += 1

            while fh_done < NWIN:
                emit_final(fh_done)
                fh_done += 1

    nc.compile()
    return nc


# ------------------------------------------------------------------ host ----
def kernel(h, X, edge_index, batch_node, weight, bias):
    h = np.asarray(h, dtype=np.float32)
    X = np.asarray(X, dtype=np.float32)
    edge_index = np.asarray(edge_index)
    weight = np.asarray(weight, dtype=np.float32)
    bias = np.asarray(bias, dtype=np.float32)

    src = edge_index[0].astype(np.int64)
    dst = edge_index[1].astype(np.int64)

    deg = np.bincount(dst, minlength=N)
    order = np.argsort(-deg, kind="stable")        # dsts by degree desc

    # deal ranks round-robin: core c gets ranks c, c+8, ...; position r//8
    dealt = order.reshape(DSTS_PER_CORE, NCORES)   # [position, core]
    # folded position order: 0, 511, 1, 510, ... balances any 128-slot run
    fold = np.empty(DSTS_PER_CORE, dtype=np.int64)
    fold[0::2] = np.arange(DSTS_PER_CORE // 2)
    fold[1::2] = DSTS_PER_CORE - 1 - np.arange(DSTS_PER_CORE // 2)
    dmat = dealt[fold, :]                          # [folded pos, core] -> dst
    degm = deg[dmat]                               # [pos, core] degrees
    cap = degm.max(axis=1)                         # shared slot capacity

    cap_key = tuple(int(x) for x in cap)
    if cap_key not in _prog_cache:
        _prog_cache[cap_key] = _build_program(cap_key)
    nc = _prog_cache[cap_key]

    s_start, nchunk, segs, pos_of_slot, islo = _shared_structure(cap)
    plan = _group_plan(nchunk)
    g0ch = plan[0]
    s_tot = int(s_start[-1])
    nslot = nchunk * 128

    # edges grouped by dst
    eorder = np.argsort(dst, kind="stable")
    estart = np.searchsorted(dst[eorder], np.arange(N))
    eend = np.searchsorted(dst[eorder], np.arange(N) + 1)

    import ml_dtypes
    hTf = np.ascontiguousarray(h.T).astype(np.float16)        # [N, C]
    # wT: lhsT rows p = ci-half lane, cols (q=(k,cih), co) -> [128, 16*256].
    # lhsT[ci, co] = weight[k][co, ci] (einsum 'kio,kon->in' contracts o=ci).
    wTb = np.ascontiguousarray(
        weight.transpose(0, 2, 1).reshape(K, 2, 128, C)       # k, cih, p, co
        .transpose(2, 0, 1, 3).reshape(128, 16 * C)).astype(ml_dtypes.bfloat16)

    bias2 = np.ascontiguousarray(bias.reshape(2, 128).T).astype(np.float32)
    iota = np.broadcast_to(np.arange(8, dtype=np.float16), (128, 8))

    # slot -> (partition, chunk)
    sl = np.arange(nslot)
    sp_ = sl % 128
    sc_ = sl // 128
    jval = (pos_of_slot - islo[np.minimum(sc_, nchunk - 1)]).astype(
        np.float16)

    in_maps = []
    perms = []
    for c in range(NCORES):
        dlist = dmat[:, c]                          # folded-order dsts
        perms.append(dlist)
        idx = np.zeros((128, nchunk * 8), dtype=np.int16)
        xr = np.zeros((128, nchunk, K), dtype=np.float16)
        dl = np.full((128, nchunk), -1.0, dtype=np.float16)

        # fill slots position by position (vectorized)
        degs_c = deg[dlist]
        fill_src = np.zeros(nslot, dtype=np.int16)
        fill_x = np.zeros((nslot, K), dtype=np.float16)
        fill_real = np.zeros(nslot, dtype=bool)
        # per position: edges of dlist[i] go to slots s_start[i]..+deg-1
        epos = np.concatenate([
            eorder[estart[d]:eend[d]] for d in dlist]) if len(dlist) else \
            np.empty(0, dtype=np.int64)
        slot_of_edge = np.concatenate([
            np.arange(s_start[i], s_start[i] + degs_c[i])
            for i in range(DSTS_PER_CORE)])
        fill_src[slot_of_edge] = src[epos].astype(np.int16)
        fill_x[slot_of_edge] = X[epos].astype(np.float16)
        fill_real[slot_of_edge] = True

        xr[sp_, sc_, :] = fill_x
        dl[sp_[fill_real[sl]], sc_[fill_real[sl]]] = jval[fill_real[sl]]

        # gather index layout per group: local j -> [j%16, j//16], tiled x8
        goff = 0
        for g, gch in enumerate(plan):
            gsl = fill_src[goff * 128:(goff + gch) * 128]
            L = gsl.shape[0]
            jj = np.arange(L)
            blk = np.zeros((16, gch * 8), dtype=np.int16)
            blk[jj % 16, jj // 16] = gsl
            idx[:, goff * 8:(goff + gch) * 8] = np.tile(blk, (8, 1))
            goff += gch

        nir = (nchunk - g0ch) * 8
        rest = np.concatenate([
            idx[:, g0ch * 8:],
            xr.reshape(128, nchunk * 8).view(np.int16),
            dl.view(np.int16),
            np.ascontiguousarray(iota).view(np.int16),
        ], axis=1)
        in_maps.append({
            "hT": hTf,
            "wT": wTb,
            "ix0": np.ascontiguousarray(idx[:, :g0ch * 8]),
            "rest": np.ascontiguousarray(rest),
            "aux": bias2,
        })

    try:
        res = run_bass_kernel_spmd(nc, in_maps, core_ids=list(range(NCORES)))
    except Exception:
        # transient device-state issues usually clear on retry
        time.sleep(10)
        res = run_bass_kernel_spmd(nc, in_maps, core_ids=list(range(NCORES)))

    out = np.empty((C, N), dtype=np.float32)
    for c in range(NCORES):
        out[:, perms[c]] = np.asarray(res.results[c]["out"],
                                      dtype=np.float32)
    return out
